# revision 1
# baseline (speedup 1.0000x reference)
"""Trainium2 Bass kernel for nn_DenseInterTripletLoss.

Strategy (validated against the reference in numpy first):
  * Shard the 9600 query cells (b=2 x n=4800) across 8 cores: core k handles
    batch k//4, cells [1280*(k%4), ...) padded to 1280 cells (10 blocks of 128).
  * Visibility of image-2 cells is computed exactly via per-cell min/max of the
    5 linear forms {ny+dd, H*dd-ny, nx+dd, W*dd-nx, dd} of the homo21 warp
    (separable over the 8x8 pixel cell), handling denominator sign flips.
  * Only ~136/4800 cells are visible -> the visible column set is compacted on
    device (gpsimd sparse_gather) and the big [rows x 4800] descriptor-distance
    min reduces to a [rows x 160] padded matmul (PE, K=65 with the per-column
    penalty folded in as an extra contraction row).
  * The 4-NN spatial-neighbor exclusion is exact: the top-4 of the separable
    coordinate distance live in a 4x4 window of nearest grid rows/cols
    (j-th row x k-th col nearest requires j*k <= 4). The 4th/5th-smallest
    window distances give a midpoint threshold tau; compacted columns with
    d2 <= tau are penalized out of the min (d2 via a K=4 matmul).
  * Bilinear descriptor sampling: the 4 corner vectors are fetched with one
    gpsimd dma_gather (indices computed on device), pos_sim assembled from
    per-row dot products and the corner Gram matrix.
  * Per-core partial (sum loss*mask, sum mask) pairs are returned; the host
    sums the 8 partials and divides (the unshard step).
"""

import hashlib
import os
import numpy as np

import concourse.bacc as bacc
import concourse.bass as bass
import concourse.mybir as mybir
import concourse.tile as tile
from concourse.bass_utils import run_bass_kernel_spmd
from concourse import library_config

F32 = mybir.dt.float32
I16 = mybir.dt.int16
U32 = mybir.dt.uint32
OP = mybir.AluOpType
AF = mybir.ActivationFunctionType
AX = mybir.AxisListType

H, W, GS = 480, 640, 8
HC, WC = 60, 80
N = HC * WC            # 4800
C = 64
NCORES = 8
CELLS = 1280           # padded cells per core
NB = 10                # blocks of 128 cells
P = 128
VCAP = 160             # compacted visible-column capacity (actual ~136)
KAUG = C + 1
BIGM = 30000.0
EPS = 1e-8
RND = 12582912.0       # 1.5*2^23, round-to-nearest trick (valid for |x| << 2^22)
INV80 = float(np.float32(1.0) / np.float32(80.0))


def _nonce_len():
    """The axon terminal caches compiled executables by an HLO hash that does
    not cover the embedded BIR, so force a distinct HLO whenever the kernel
    source (or build-mode env) changes by varying a dummy input's shape."""
    h = hashlib.sha1()
    with open(os.path.abspath(__file__), "rb") as f:
        h.update(f.read())
    h.update(os.environ.get("KERNEL_STAGE", "").encode())
    h.update(os.environ.get("KERNEL_DEBUG", "").encode())
    h.update(os.environ.get("KERNEL_GDBG", "").encode())
    h.update(os.environ.get("KERNEL_NODOTS", "").encode())
    return 1 + int(h.hexdigest(), 16) % 997


def _emit(tc):
    nc = tc.nc
    dr = {}

    def din(name, shape, dtype=F32):
        dr[name] = nc.dram_tensor(name, list(shape), dtype, kind="ExternalInput").ap()
        return dr[name]

    def dscr(name, shape, dtype=F32):
        dr[name] = nc.dram_tensor(name, list(shape), dtype, kind="Internal").ap()
        return dr[name]

    desc2t = din("desc2t", (N, C))
    d1m = din("d1m", (C, CELLS))
    d1t = din("d1t", (CELLS, C))
    coords = din("coords", (3, CELLS))
    homot = din("homot", (3, 3))
    h21 = din("h21", (1, 9))
    pxv = din("pxv", (1, W))
    pyv = din("pyv", (1, H))
    validm = din("validm", (P, NB))
    ident = din("ident", (P, P))
    idxg1 = din("idxg1", (HC, WC))
    onesv = din("onesv", (1, VCAP))
    serid = din("serid", (16, VCAP // 16))
    din("nonce", (1, _nonce_len()))
    out_part = nc.dram_tensor("partial", [2], F32, kind="ExternalOutput").ap()
    DBG = os.environ.get("KERNEL_DEBUG") == "1"
    dbg = {}
    if DBG:
        for nm in ("d_pos", "d_neg", "d_negsq", "d_yp", "d_xp", "d_tau",
                   "d_num", "d_nsq", "d_match", "d_rl",
                   "d_w00", "d_w01", "d_w10", "d_w11",
                   "d_i00", "d_i01", "d_i10", "d_i11", "d_y0f", "d_wy1"):
            dbg[nm] = nc.dram_tensor(nm, [P, NB], F32, kind="ExternalOutput").ap()
        for nm in ("d_pdots", "d_grams"):
            pass
        dbg["d_pdots"] = nc.dram_tensor("d_pdots", [P, NB, 4], F32, kind="ExternalOutput").ap()
        dbg["d_grams"] = nc.dram_tensor("d_grams", [P, NB, 10], F32, kind="ExternalOutput").ap()

    scr_marker = dscr("scr_marker", (HC, WC))
    scr_rows = dscr("scr_rows", (4, VCAP))
    scr_corner = dscr("scr_corner", (NB, 4, P), I16)
    scr_vidx = dscr("scr_vidx", (16, VCAP // 16), I16)

    v = nc.vector
    s = nc.scalar
    g = nc.gpsimd
    te = nc.tensor
    STAGE = int(os.environ.get("KERNEL_STAGE", "9"))

    with (
        tc.tile_pool(name="cp", bufs=1) as cp,
        tc.tile_pool(name="wp", bufs=3) as wp,
    ):
        # ---------------- load persistent inputs ----------------
        sb_d1m = cp.tile([C, CELLS], F32, tag="d1m")
        nc.sync.dma_start(sb_d1m[:], d1m[:])
        sb_d1s = cp.tile([C, CELLS], F32, tag="d1s")
        s.mul(sb_d1s[:], sb_d1m[:], -2.0)
        sb_coords = cp.tile([3, CELLS], F32, tag="coords")
        nc.sync.dma_start(sb_coords[:], coords[:])
        sb_homot = cp.tile([3, 3], F32, tag="homot")
        nc.sync.dma_start(sb_homot[:], homot[:])
        sb_h21 = cp.tile([1, 9], F32, tag="h21")
        nc.sync.dma_start(sb_h21[:], h21[:])
        sb_pxv = cp.tile([1, W], F32, tag="pxv")
        nc.sync.dma_start(sb_pxv[:], pxv[:])
        sb_pyv = cp.tile([1, H], F32, tag="pyv")
        nc.sync.dma_start(sb_pyv[:], pyv[:])
        sb_validm = cp.tile([P, NB], F32, tag="validm")
        nc.sync.dma_start(sb_validm[:], validm[:])
        sb_ident = cp.tile([P, P], F32, tag="ident")
        nc.sync.dma_start(sb_ident[:], ident[:])
        sb_idxg1 = cp.tile([HC, WC], F32, tag="idxg1")
        nc.sync.dma_start(sb_idxg1[:], idxg1[:])
        sb_ones = cp.tile([1, VCAP], F32, tag="onesv")
        nc.sync.dma_start(sb_ones[:], onesv[:])
        sb_d1tb = cp.tile([P, NB, C], F32, tag="d1tb")
        nc.sync.dma_start(
            sb_d1tb[:], d1t[:].rearrange("(t p) c -> p t c", p=P)
        )

        # ---------------- visibility (per image-2 cell) ----------------
        # linear forms L(x,y) = ax*x + (vy(y)); exact per-cell min/max via
        # separable 8-pixel min/max on each axis, combined with a K=2 matmul.
        gg = sb_h21
        e = lambda i, j: gg[0:1, 3 * i + j : 3 * i + j + 1]

        c22 = cp.tile([1, 1], F32, tag="c22")
        v.tensor_single_scalar(c22[:], e(2, 2), EPS, OP.add)
        nyv = cp.tile([1, H], F32, tag="nyv")
        v.tensor_scalar(nyv[:], sb_pyv[:], e(1, 1), e(1, 2), OP.mult, OP.add)
        nxv = cp.tile([1, H], F32, tag="nxv")
        v.tensor_scalar(nxv[:], sb_pyv[:], e(0, 1), e(0, 2), OP.mult, OP.add)
        ddv = cp.tile([1, H], F32, tag="ddv")
        v.tensor_scalar(ddv[:], sb_pyv[:], e(2, 1), c22[:], OP.mult, OP.add)

        # x coefficients
        axA = cp.tile([1, 1], F32, tag="axA")
        v.tensor_tensor(axA[:], e(1, 0), e(2, 0), OP.add)
        axC = cp.tile([1, 1], F32, tag="axC")
        v.tensor_tensor(axC[:], e(0, 0), e(2, 0), OP.add)
        tB = cp.tile([1, 1], F32, tag="tB")
        v.tensor_single_scalar(tB[:], e(2, 0), float(H), OP.mult)
        axB = cp.tile([1, 1], F32, tag="axB")
        v.tensor_tensor(axB[:], tB[:], e(1, 0), OP.subtract)
        tD = cp.tile([1, 1], F32, tag="tD")
        v.tensor_single_scalar(tD[:], e(2, 0), float(W), OP.mult)
        axD = cp.tile([1, 1], F32, tag="axD")
        v.tensor_tensor(axD[:], tD[:], e(0, 0), OP.subtract)

        # y-side vectors
        Av = cp.tile([1, H], F32, tag="Av")
        v.tensor_tensor(Av[:], nyv[:], ddv[:], OP.add)
        Bv = cp.tile([1, H], F32, tag="Bv")
        v.tensor_scalar(Bv[:], ddv[:], float(H), None, OP.mult)
        v.tensor_tensor(Bv[:], Bv[:], nyv[:], OP.subtract)
        Cv = cp.tile([1, H], F32, tag="Cv")
        v.tensor_tensor(Cv[:], nxv[:], ddv[:], OP.add)
        Dv = cp.tile([1, H], F32, tag="Dv")
        v.tensor_scalar(Dv[:], ddv[:], float(W), None, OP.mult)
        v.tensor_tensor(Dv[:], Dv[:], nxv[:], OP.subtract)

        def cell_extreme(ppool, axc, vvec, op, tagp):
            """per-cell extreme (min or max) of axc*px + vvec(py) -> PSUM [60,80]"""
            ux = wp.tile([1, W], F32, tag="ux")
            v.tensor_single_scalar(ux[:], sb_pxv[:], axc[:], OP.mult)
            ue = wp.tile([1, WC], F32, tag="ue")
            v.tensor_reduce(ue[:], ux[:].rearrange("p (a b) -> p a b", b=GS), AX.X, op)
            ve = wp.tile([1, HC], F32, tag="ve")
            v.tensor_reduce(ve[:], vvec[:].rearrange("p (a b) -> p a b", b=GS), AX.X, op)
            ps = ppool.tile([HC, WC], F32, tag=tagp)
            te.matmul(ps[:], ve[:, 0:HC], sb_ones[:, 0:WC], start=True, stop=False)
            te.matmul(ps[:], sb_ones[:, 0:HC], ue[:, 0:WC], start=False, stop=True)
            return ps

        with tc.tile_pool(name="ppv", bufs=3, space=bass.MemorySpace.PSUM) as ppv:
            mins = [
                cell_extreme(ppv, axA, Av, OP.min, "vmin"),
                cell_extreme(ppv, axB, Bv, OP.min, "vmin"),
                cell_extreme(ppv, axC, Cv, OP.min, "vmin"),
                cell_extreme(ppv, axD, Dv, OP.min, "vmin"),
                cell_extreme(ppv, e(2, 0), ddv, OP.min, "vmin"),
            ]
            mpos = cp.tile([HC, WC], F32, tag="mpos")
            s.copy(mpos[:], mins[0][:])
            for t_ in mins[1:]:
                v.tensor_tensor(mpos[:], mpos[:], t_[:], OP.min)
            maxs = [
                cell_extreme(ppv, axA, Av, OP.max, "vmax"),
                cell_extreme(ppv, axB, Bv, OP.max, "vmax"),
                cell_extreme(ppv, axC, Cv, OP.max, "vmax"),
                cell_extreme(ppv, axD, Dv, OP.max, "vmax"),
                cell_extreme(ppv, e(2, 0), ddv, OP.max, "vmax"),
            ]
            mneg = cp.tile([HC, WC], F32, tag="mneg")
            s.copy(mneg[:], maxs[0][:])
            for t_ in maxs[1:]:
                v.tensor_tensor(mneg[:], mneg[:], t_[:], OP.max)
        vpos = cp.tile([HC, WC], F32, tag="vpos")
        v.tensor_single_scalar(vpos[:], mpos[:], 0.0, OP.is_gt)
        vneg = cp.tile([HC, WC], F32, tag="vneg")
        v.tensor_single_scalar(vneg[:], mneg[:], 0.0, OP.is_lt)
        vis01 = cp.tile([HC, WC], F32, tag="vis01")
        v.tensor_tensor(vis01[:], vpos[:], vneg[:], OP.add)
        marker = cp.tile([HC, WC], F32, tag="marker")
        v.tensor_tensor(marker[:], vis01[:], sb_idxg1[:], OP.mult)
        v.tensor_scalar(marker[:], marker[:], 1.0, None, OP.subtract)

        # serialize marker into sparse_gather's [16, 300] wrapped layout
        nc.sync.dma_start(scr_marker[:], marker[:])
        sp_in = cp.tile([16, N // 16], F32, tag="sp_in")
        nc.sync.dma_start(
            sp_in[:], scr_marker[:].rearrange("r (cf cp) -> cp (r cf)", cp=16)
        )
        sp_out = cp.tile([16, VCAP // 16], F32, tag="sp_out")
        sp_nf = cp.tile([1, 1], U32, tag="sp_nf")
        with tc.tile_critical():
            g.sparse_gather(sp_out[:], sp_in[:], num_found=sp_nf[:])
        sb_serid = cp.tile([16, VCAP // 16], F32, tag="serid")
        nc.sync.dma_start(sb_serid[:], serid[:])
        nf32 = cp.tile([1, 1], F32, tag="nf32")
        v.tensor_copy(nf32[:], sp_nf[:])
        nfp = None
        with tc.tile_pool(name="ppn", bufs=1, space=bass.MemorySpace.PSUM) as ppn:
            nfps = ppn.tile([16, 1], F32, tag="nfps")
            te.matmul(nfps[:], sb_ones[0:1, 0:16], nf32[:])
            nfb = cp.tile([16, 1], F32, tag="nfb")
            s.copy(nfb[:], nfps[:])
        padm = cp.tile([16, VCAP // 16], mybir.dt.uint8, tag="padm")
        v.tensor_single_scalar(padm[:], sb_serid[:], nfb[:], OP.is_lt)
        spc = cp.tile([16, VCAP // 16], F32, tag="spc")
        v.memset(spc[:], -1.0)
        v.copy_predicated(spc[:], padm[:], sp_out[:])

        # ---------------- compacted visible columns ----------------
        sp0 = cp.tile([16, VCAP // 16], F32, tag="sp0")
        v.tensor_single_scalar(sp0[:], spc[:], 0.0, OP.max)
        idxs16a = cp.tile([16, VCAP // 16], I16, tag="idxs16a")
        v.tensor_copy(idxs16a[:], sp0[:])
        nc.sync.dma_start(scr_vidx[:], idxs16a[:])
        idxs16r = cp.tile([P, VCAP // 16], I16, tag="idxs16r")
        for rr_ in range(8):
            nc.sync.dma_start(idxs16r[16 * rr_ : 16 * rr_ + 16, :], scr_vidx[:])
        idxs16 = cp.tile([P, VCAP // 16], I16, tag="idxs16")
        v.tensor_copy(idxs16[:], idxs16r[:])

        visraw = cp.tile([P, 2, C], F32, tag="visraw")
        gsem = nc.alloc_semaphore("gsem")
        with tc.tile_critical():
            g.dma_gather(
                visraw[:], desc2t[:], idxs16[:],
                num_idxs=VCAP, num_idxs_reg=VCAP, elem_size=C,
            ).then_inc(gsem, 16)
            g.wait_ge(gsem, 16)

        sb_rhsS = cp.tile([KAUG, VCAP], F32, tag="rhsS")
        with tc.tile_pool(name="ppt", bufs=2, space=bass.MemorySpace.PSUM) as ppt:
            for q in range(2):
                pt = ppt.tile([C, P], F32, tag="ptr")
                te.transpose(pt[:], visraw[:, q, :], sb_ident[:])
                w_ = P if q == 0 else VCAP - P
                s.copy(sb_rhsS[0:C, q * P : q * P + w_], pt[:, 0:w_])

        # penalty row: 2 for valid slots, 2+BIGM for pads (sparse pads are -1)
        r64 = cp.tile([16, VCAP // 16], F32, tag="r64")
        v.tensor_scalar(r64[:], spc[:], 0.0, BIGM, OP.is_lt, OP.mult)
        v.tensor_scalar(r64[:], r64[:], 2.0, None, OP.add)
        nc.sync.dma_start(
            scr_rows[0:1, :].rearrange("o (f p) -> o p f", p=16), r64[:]
        )
        nc.sync.dma_start(sb_rhsS[C : C + 1, :], scr_rows[0:1, :])

        # d2c rhs rows {-2yj, -2xj, 1, b2j} derived from compacted indices
        uu = cp.tile([16, VCAP // 16], F32, tag="uu")
        v.tensor_scalar(uu[:], spc[:], 0.5, INV80, OP.add, OP.mult)
        rr0 = cp.tile([16, VCAP // 16], F32, tag="rr0")
        v.tensor_scalar(rr0[:], uu[:], RND, RND, OP.add, OP.subtract)
        rgt = cp.tile([16, VCAP // 16], F32, tag="rgt")
        v.tensor_tensor(rgt[:], rr0[:], uu[:], OP.is_gt)
        rj = cp.tile([16, VCAP // 16], F32, tag="rj")
        v.tensor_tensor(rj[:], rr0[:], rgt[:], OP.subtract)
        cj = cp.tile([16, VCAP // 16], F32, tag="cj")
        v.tensor_scalar(cj[:], rj[:], 80.0, None, OP.mult)
        v.tensor_tensor(cj[:], spc[:], cj[:], OP.subtract)
        rw0 = cp.tile([16, VCAP // 16], F32, tag="rw0")
        v.tensor_scalar(rw0[:], rj[:], -16.0, -7.0, OP.mult, OP.add)
        rw1 = cp.tile([16, VCAP // 16], F32, tag="rw1")
        v.tensor_scalar(rw1[:], cj[:], -16.0, -7.0, OP.mult, OP.add)
        yj = cp.tile([16, VCAP // 16], F32, tag="yj")
        v.tensor_scalar(yj[:], rj[:], 8.0, 3.5, OP.mult, OP.add)
        xj = cp.tile([16, VCAP // 16], F32, tag="xj")
        v.tensor_scalar(xj[:], cj[:], 8.0, 3.5, OP.mult, OP.add)
        b2j = cp.tile([16, VCAP // 16], F32, tag="b2j")
        v.tensor_tensor(b2j[:], yj[:], yj[:], OP.mult)
        xj2 = cp.tile([16, VCAP // 16], F32, tag="xj2")
        v.tensor_tensor(xj2[:], xj[:], xj[:], OP.mult)
        v.tensor_tensor(b2j[:], b2j[:], xj2[:], OP.add)

        sb_d2crhs = cp.tile([4, VCAP], F32, tag="d2crhs")
        for i, tl in ((1, rw0), (2, rw1), (3, b2j)):
            nc.sync.dma_start(
                scr_rows[i : i + 1, :].rearrange("o (f p) -> o p f", p=16), tl[:]
            )
        nc.sync.dma_start(sb_d2crhs[0:1, :], scr_rows[1:2, :])
        nc.sync.dma_start(sb_d2crhs[1:2, :], scr_rows[2:3, :])
        nc.sync.dma_start(sb_d2crhs[2:3, :], onesv[:])
        nc.sync.dma_start(sb_d2crhs[3:4, :], scr_rows[3:4, :])

        if STAGE <= 1:
            ones1 = cp.tile([P, 1], F32, tag="ones1")
            v.memset(ones1[:], 1.0)
            sums2 = cp.tile([P, 2], F32, tag="sums2")
            v.tensor_copy(sums2[0:KAUG, 0:1], sb_rhsS[:, 0:1])
            v.tensor_copy(sums2[0:4, 1:2], sb_d2crhs[:, 0:1])
            psb = cp.tile([2, 1], F32, tag="psb")
            v.tensor_copy(psb[:], sums2[0:2, 0:1])
            nc.sync.dma_start(out_part[:].rearrange("(a b) -> a b", b=1), psb[:])
            return dr

        # ---------------- warp (both layouts) ----------------
        yp = cp.tile([P, NB], F32, tag="yp")
        xp = cp.tile([P, NB], F32, tag="xp")
        lhsTd2c = []
        ppw_cm = tc.tile_pool(name="ppw", bufs=2, space=bass.MemorySpace.PSUM)
        ppw = ppw_cm.__enter__()
        for t in range(NB):
            qp = ppw.tile([P, 3], F32, tag="qp")
            te.matmul(qp[:], sb_coords[:, bass.ts(t, P)], sb_homot[:])
            zc = wp.tile([P, 1], F32, tag="zc")
            v.tensor_scalar(zc[:], qp[:, 2:3], EPS, None, OP.add)
            rz = wp.tile([P, 1], F32, tag="rz")
            v.reciprocal(rz[:], zc[:])
            v.tensor_tensor(yp[:, t : t + 1], qp[:, 1:2], rz[:], OP.mult)
            v.tensor_tensor(xp[:, t : t + 1], qp[:, 0:1], rz[:], OP.mult)

            quad = wp.tile([P, 4], F32, tag="quad")
            v.tensor_copy(quad[:, 0:1], yp[:, t : t + 1])
            v.tensor_copy(quad[:, 1:2], xp[:, t : t + 1])
            v.tensor_tensor(quad[:, 2:3], yp[:, t : t + 1], yp[:, t : t + 1], OP.mult)
            x2c = wp.tile([P, 1], F32, tag="x2c")
            v.tensor_tensor(x2c[:], xp[:, t : t + 1], xp[:, t : t + 1], OP.mult)
            v.tensor_tensor(quad[:, 2:3], quad[:, 2:3], x2c[:], OP.add)
            v.memset(quad[:, 3:4], 1.0)
            ldp = ppw.tile([4, P], F32, tag="qk")
            te.transpose(ldp[:], quad[:], sb_ident[:])
            ld = cp.tile([4, P], F32, tag=f"lhsTd2c{t}")
            s.copy(ld[:], ldp[:])
            lhsTd2c.append(ld)
        ppw_cm.__exit__(None, None, None)

        if STAGE <= 2:
            sums2 = cp.tile([P, 2], F32, tag="sums2")
            v.tensor_copy(sums2[:, 0:1], yp[:, 0:1])
            v.tensor_copy(sums2[:, 1:2], xp[:, 0:1])
            psb = cp.tile([2, 1], F32, tag="psb")
            v.tensor_copy(psb[:], sums2[0:2, 0:1])
            nc.sync.dma_start(out_part[:].rearrange("(a b) -> a b", b=1), psb[:])
            return dr

        # ---------------- per-cell floors / windows / weights ----------------
        def floor_of(src, tag):
            r0 = cp.tile([P, NB], F32, tag=tag + "_r0")
            v.tensor_scalar(r0[:], src[:], RND, RND, OP.add, OP.subtract)
            gt_ = cp.tile([P, NB], F32, tag=tag + "_gt")
            v.tensor_tensor(gt_[:], r0[:], src[:], OP.is_gt)
            fl = cp.tile([P, NB], F32, tag=tag)
            v.tensor_tensor(fl[:], r0[:], gt_[:], OP.subtract)
            return fl

        dgy = cp.tile([P, NB], F32, tag="dgy")
        v.tensor_scalar(dgy[:], yp[:], -4.0, 0.5, OP.add, OP.add)
        v.tensor_scalar(dgy[:], dgy[:], 0.125, None, OP.mult)
        dgx = cp.tile([P, NB], F32, tag="dgx")
        v.tensor_scalar(dgx[:], xp[:], -4.0, 0.5, OP.add, OP.add)
        v.tensor_scalar(dgx[:], dgx[:], 0.125, None, OP.mult)

        y0f = floor_of(dgy, "y0f")
        x0f = floor_of(dgx, "x0f")
        wy1 = cp.tile([P, NB], F32, tag="wy1")
        v.tensor_tensor(wy1[:], dgy[:], y0f[:], OP.subtract)
        wx1 = cp.tile([P, NB], F32, tag="wx1")
        v.tensor_tensor(wx1[:], dgx[:], x0f[:], OP.subtract)

        def clampf(src, lo, hi, tag):
            t_ = cp.tile([P, NB], F32, tag=tag)
            v.tensor_scalar(t_[:], src[:], lo, hi, OP.max, OP.min)
            return t_

        y0c = clampf(y0f, 0.0, float(HC - 1), "y0c")
        x0c = clampf(x0f, 0.0, float(WC - 1), "x0c")
        y1f = cp.tile([P, NB], F32, tag="y1f")
        v.tensor_scalar(y1f[:], y0f[:], 1.0, None, OP.add)
        x1f = cp.tile([P, NB], F32, tag="x1f")
        v.tensor_scalar(x1f[:], x0f[:], 1.0, None, OP.add)
        y1c = clampf(y1f, 0.0, float(HC - 1), "y1c")
        x1c = clampf(x1f, 0.0, float(WC - 1), "x1c")

        def iseq(a, b, tag):
            t_ = cp.tile([P, NB], F32, tag=tag)
            v.tensor_tensor(t_[:], a[:], b[:], OP.is_equal)
            return t_

        vy0 = iseq(y0f, y0c, "vy0")
        vy1 = iseq(y1f, y1c, "vy1")
        vx0 = iseq(x0f, x0c, "vx0")
        vx1 = iseq(x1f, x1c, "vx1")

        w4r = cp.tile([P, NB], F32, tag="w4r")
        v.tensor_scalar(w4r[:], y0f[:], -1.0, 0.0, OP.add, OP.max)
        v.tensor_scalar(w4r[:], w4r[:], float(HC - 4), None, OP.min)
        w4c = cp.tile([P, NB], F32, tag="w4c")
        v.tensor_scalar(w4c[:], x0f[:], -1.0, 0.0, OP.add, OP.max)
        v.tensor_scalar(w4c[:], w4c[:], float(WC - 4), None, OP.min)

        # negated squared axis distances for the 4x4 window
        ndy2 = cp.tile([P, NB, 4], F32, tag="ndy2")
        ndx2 = cp.tile([P, NB, 4], F32, tag="ndx2")
        for a in range(4):
            ty = wp.tile([P, NB], F32, tag="wty")
            v.tensor_scalar(ty[:], w4r[:], 8.0, 3.5 + 8.0 * a, OP.mult, OP.add)
            v.tensor_tensor(ty[:], yp[:], ty[:], OP.subtract)
            v.tensor_tensor(ty[:], ty[:], ty[:], OP.mult)
            v.tensor_scalar(ndy2[:, :, a], ty[:], -1.0, None, OP.mult)
            tx = wp.tile([P, NB], F32, tag="wtx")
            v.tensor_scalar(tx[:], w4c[:], 8.0, 3.5 + 8.0 * a, OP.mult, OP.add)
            v.tensor_tensor(tx[:], xp[:], tx[:], OP.subtract)
            v.tensor_tensor(tx[:], tx[:], tx[:], OP.mult)
            v.tensor_scalar(ndx2[:, :, a], tx[:], -1.0, None, OP.mult)
        nd2w = cp.tile([P, NB, 16], F32, tag="nd2w")
        for a in range(4):
            for b in range(4):
                v.tensor_tensor(
                    nd2w[:, :, 4 * a + b], ndy2[:, :, a], ndx2[:, :, b], OP.add
                )

        tau = cp.tile([P, NB], F32, tag="tau")
        for t in range(NB):
            m8 = wp.tile([P, 8], F32, tag="m8")
            v.max(m8[:], nd2w[:, t, :])
            s34 = wp.tile([P, 1], F32, tag="s34")
            v.tensor_tensor(s34[:], m8[:, 3:4], m8[:, 4:5], OP.add)
            v.tensor_scalar(tau[:, t : t + 1], s34[:], -0.5, None, OP.mult)

        if STAGE <= 3:
            sums2 = cp.tile([P, 2], F32, tag="sums2")
            v.tensor_copy(sums2[:, 0:1], tau[:, 0:1])
            v.tensor_copy(sums2[:, 1:2], y0f[:, 0:1])
            psb = cp.tile([2, 1], F32, tag="psb")
            v.tensor_copy(psb[:], sums2[0:2, 0:1])
            nc.sync.dma_start(out_part[:].rearrange("(a b) -> a b", b=1), psb[:])
            return dr

        # ---------------- corner indices -> gather ----------------
        i00 = cp.tile([P, NB], F32, tag="i00")
        v.tensor_scalar(i00[:], y0c[:], 80.0, None, OP.mult)
        i01 = cp.tile([P, NB], F32, tag="i01")
        v.tensor_tensor(i01[:], i00[:], x1c[:], OP.add)
        v.tensor_tensor(i00[:], i00[:], x0c[:], OP.add)
        i10 = cp.tile([P, NB], F32, tag="i10")
        v.tensor_scalar(i10[:], y1c[:], 80.0, None, OP.mult)
        i11 = cp.tile([P, NB], F32, tag="i11")
        v.tensor_tensor(i11[:], i10[:], x1c[:], OP.add)
        v.tensor_tensor(i10[:], i10[:], x0c[:], OP.add)

        for k, tl in enumerate((i00, i01, i10, i11)):
            ci = wp.tile([P, NB], I16, tag="ci")
            v.tensor_copy(ci[:], tl[:])
            nc.sync.dma_start(scr_corner[:, k, :].rearrange("t p -> p t"), ci[:])
        cgidx = cp.tile([P, 4 * P * NB // 16], I16, tag="cgidx")
        for rr_ in range(8):
            nc.sync.dma_start(
                cgidx[16 * rr_ : 16 * rr_ + 16, :],
                scr_corner[:].rearrange("t k p -> (t k p)").rearrange(
                    "(f pp) -> pp f", pp=16
                ),
            )
        if STAGE <= 4:
            d_cg = nc.dram_tensor("d_cgidx", [P, 4 * P * NB // 16], I16,
                                  kind="ExternalOutput").ap()
            nc.sync.dma_start(d_cg[:], cgidx[:])
            cgf = cp.tile([16, 4 * P * NB // 16], F32, tag="cgf")
            v.tensor_copy(cgf[:], cgidx[0:16, :])
            sums2 = cp.tile([P, 2], F32, tag="sums2")
            v.tensor_copy(sums2[0:16, 0:1], cgf[:, 0:1])
            v.tensor_copy(sums2[0:16, 1:2], cgf[:, 1:2])
            psb = cp.tile([2, 1], F32, tag="psb")
            v.tensor_copy(psb[:], sums2[0:2, 0:1])
            nc.sync.dma_start(out_part[:].rearrange("(a b) -> a b", b=1), psb[:])
            return dr

        if os.environ.get("KERNEL_GDBG") == "zero":
            v.memset(cgidx[:], 0)
        cgidx2 = cp.tile([P, 4 * P * NB // 16], I16, tag="cgidx2")
        v.tensor_copy(cgidx2[:], cgidx[:])
        visCorn = cp.tile([P, 4 * NB, C], F32, tag="visCorn")
        gsem2 = nc.alloc_semaphore("gsem2")
        NSPLIT = 8
        QS = 4 * NB // NSPLIT
        with tc.tile_critical():
            for j in range(NSPLIT):
                g.dma_gather(
                    visCorn[:, QS * j : QS * (j + 1), :], desc2t[:],
                    cgidx2[:, QS * 8 * j : QS * 8 * (j + 1)],
                    num_idxs=P * QS, num_idxs_reg=P * QS, elem_size=C,
                ).then_inc(gsem2, 16)
            g.wait_ge(gsem2, 16 * NSPLIT)

        if STAGE <= 5:
            sums2 = cp.tile([P, 2], F32, tag="sums2")
            v.tensor_copy(sums2[:, 0:1], visCorn[:, 0, 0:1])
            v.tensor_copy(sums2[:, 1:2], tau[:, 0:1])
            psb = cp.tile([2, 1], F32, tag="psb")
            v.tensor_copy(psb[:], sums2[0:2, 0:1])
            nc.sync.dma_start(out_part[:].rearrange("(a b) -> a b", b=1), psb[:])
            return dr

        # ---------------- main per-block loop ----------------
        negsq = cp.tile([P, NB], F32, tag="negsq")
        pdots = cp.tile([P, NB, 4], F32, tag="pdots")
        PAIRS = [(0, 0), (0, 1), (0, 2), (0, 3), (1, 1), (1, 2), (1, 3),
                 (2, 2), (2, 3), (3, 3)]
        grams = cp.tile([P, NB, len(PAIRS)], F32, tag="grams")

        ppm_cm = tc.tile_pool(name="ppm", bufs=3, space=bass.MemorySpace.PSUM)
        ppm = ppm_cm.__enter__()
        for t in range(NB):
            lhsTS = wp.tile([KAUG, P], F32, tag="lhsTS")
            s.copy(lhsTS[0:C, :], sb_d1s[:, bass.ts(t, P)])
            v.memset(lhsTS[C : C + 1, :], 1.0)
            tps = ppm.tile([P, VCAP], F32, tag="tps")
            te.matmul(tps[:], lhsTS[:], sb_rhsS[:])
            dps = ppm.tile([P, VCAP], F32, tag="dps")
            te.matmul(dps[:], lhsTd2c[t][:], sb_d2crhs[:])
            pen = wp.tile([P, VCAP], F32, tag="pen")
            v.tensor_scalar(pen[:], dps[:], tau[:, t : t + 1], BIGM, OP.is_le, OP.mult)
            tfin = wp.tile([P, VCAP], F32, tag="tfin")
            v.tensor_tensor(tfin[:], tps[:], pen[:], OP.add)
            v.tensor_reduce(negsq[:, t : t + 1], tfin[:], AX.X, OP.min)

            dsc = wp.tile([P, C], F32, tag="dsc")
            for k in range(4):
                v.tensor_tensor(dsc[:], sb_d1tb[:, t, :], visCorn[:, 4 * t + k, :], OP.mult)
                v.tensor_reduce(pdots[:, t, k : k + 1], dsc[:], AX.X, OP.add)
            for pi, (k, l) in enumerate(PAIRS):
                v.tensor_tensor(dsc[:], visCorn[:, 4 * t + k, :], visCorn[:, 4 * t + l, :], OP.mult)
                v.tensor_reduce(grams[:, t, pi : pi + 1], dsc[:], AX.X, OP.add)
        ppm_cm.__exit__(None, None, None)

        if STAGE <= 6:
            sums2 = cp.tile([P, 2], F32, tag="sums2")
            v.tensor_copy(sums2[:, 0:1], negsq[:, 0:1])
            v.tensor_copy(sums2[:, 1:2], pdots[:, 0, 0:1])
            psb = cp.tile([2, 1], F32, tag="psb")
            v.tensor_copy(psb[:], sums2[0:2, 0:1])
            nc.sync.dma_start(out_part[:].rearrange("(a b) -> a b", b=1), psb[:])
            return dr

        # ---------------- pos / neg / loss ----------------
        u1 = cp.tile([P, NB], F32, tag="u1")  # 1-wy1
        v.tensor_scalar(u1[:], wy1[:], -1.0, 1.0, OP.mult, OP.add)
        u2 = cp.tile([P, NB], F32, tag="u2")  # 1-wx1
        v.tensor_scalar(u2[:], wx1[:], -1.0, 1.0, OP.mult, OP.add)

        # corner weights w_k = wgt_y * wgt_x * valid_y * valid_x
        def mkw(wy_, vy_, wx_, vx_, tag):
            t_ = cp.tile([P, NB], F32, tag=tag)
            v.tensor_tensor(t_[:], wy_[:], wx_[:], OP.mult)
            vv = wp.tile([P, NB], F32, tag="vv")
            v.tensor_tensor(vv[:], vy_[:], vx_[:], OP.mult)
            v.tensor_tensor(t_[:], t_[:], vv[:], OP.mult)
            return t_

        w00 = mkw(u1, vy0, u2, vx0, "w00")
        w01 = mkw(u1, vy0, wx1, vx1, "w01")
        w10 = mkw(wy1, vy1, u2, vx0, "w10")
        w11 = mkw(wy1, vy1, wx1, vx1, "w11")
        wk = [w00, w01, w10, w11]

        num = cp.tile([P, NB], F32, tag="num")
        v.tensor_tensor(num[:], wk[0][:], pdots[:, :, 0], OP.mult)
        for k in range(1, 4):
            tk = wp.tile([P, NB], F32, tag="tk")
            v.tensor_tensor(tk[:], wk[k][:], pdots[:, :, k], OP.mult)
            v.tensor_tensor(num[:], num[:], tk[:], OP.add)

        nsq = cp.tile([P, NB], F32, tag="nsq")
        first = True
        for pi, (k, l) in enumerate(PAIRS):
            co = wp.tile([P, NB], F32, tag="co")
            v.tensor_tensor(co[:], wk[k][:], wk[l][:], OP.mult)
            if k != l:
                v.tensor_scalar(co[:], co[:], 2.0, None, OP.mult)
            v.tensor_tensor(co[:], co[:], grams[:, :, pi], OP.mult)
            if first:
                v.tensor_copy(nsq[:], co[:])
                first = False
            else:
                v.tensor_tensor(nsq[:], nsq[:], co[:], OP.add)
        v.tensor_scalar(nsq[:], nsq[:], 0.0, None, OP.max)
        nrm = cp.tile([P, NB], F32, tag="nrm")
        s.sqrt(nrm[:], nsq[:])
        v.tensor_scalar(nrm[:], nrm[:], EPS, None, OP.add)
        rden = cp.tile([P, NB], F32, tag="rden")
        v.reciprocal(rden[:], nrm[:])
        posd = cp.tile([P, NB], F32, tag="posd")
        v.tensor_tensor(posd[:], num[:], rden[:], OP.mult)
        pa = cp.tile([P, NB], F32, tag="pa")
        v.tensor_scalar(pa[:], posd[:], -2.0, 2.0, OP.mult, OP.add)
        v.tensor_scalar(pa[:], pa[:], EPS, None, OP.max)
        pos = cp.tile([P, NB], F32, tag="pos")
        s.sqrt(pos[:], pa[:])

        ngc = cp.tile([P, NB], F32, tag="ngc")
        v.tensor_scalar(ngc[:], negsq[:], EPS, None, OP.max)
        neg = cp.tile([P, NB], F32, tag="neg")
        s.sqrt(neg[:], ngc[:])

        # match mask
        m1 = cp.tile([P, NB], F32, tag="m1")
        v.tensor_single_scalar(m1[:], yp[:], 0.0, OP.is_ge)
        m2 = cp.tile([P, NB], F32, tag="m2")
        v.tensor_single_scalar(m2[:], yp[:], float(H - 1), OP.is_le)
        v.tensor_tensor(m1[:], m1[:], m2[:], OP.mult)
        v.tensor_single_scalar(m2[:], xp[:], 0.0, OP.is_ge)
        v.tensor_tensor(m1[:], m1[:], m2[:], OP.mult)
        v.tensor_single_scalar(m2[:], xp[:], float(W - 1), OP.is_le)
        v.tensor_tensor(m1[:], m1[:], m2[:], OP.mult)
        v.tensor_tensor(m1[:], m1[:], sb_validm[:], OP.mult)

        lt = cp.tile([P, NB], F32, tag="lt")
        v.tensor_tensor(lt[:], pos[:], neg[:], OP.subtract)
        rl = cp.tile([P, NB], F32, tag="rl")
        s.activation(rl[:], lt[:], AF.Relu, bias=1.0, scale=1.0)
        rm = cp.tile([P, NB], F32, tag="rm")
        v.tensor_tensor(rm[:], rl[:], m1[:], OP.mult)

        sums2 = cp.tile([P, 2], F32, tag="sums2")
        lsc = cp.tile([P, NB], F32, tag="lsc")
        v.tensor_tensor(lsc[:], rm[:], rl[:], OP.mult)
        v.tensor_reduce(sums2[:, 0:1], lsc[:], AX.X, OP.add)
        v.tensor_reduce(sums2[:, 1:2], m1[:], AX.X, OP.add)
        ones1 = cp.tile([P, 1], F32, tag="ones1")
        v.memset(ones1[:], 1.0)
        if DBG:
            for nm, tl in (("d_pos", pos), ("d_neg", neg), ("d_negsq", negsq),
                           ("d_yp", yp), ("d_xp", xp), ("d_tau", tau),
                           ("d_num", num), ("d_nsq", nsq), ("d_match", m1),
                           ("d_rl", rl), ("d_w00", w00), ("d_w01", w01),
                           ("d_w10", w10), ("d_w11", w11), ("d_i00", i00),
                           ("d_i01", i01), ("d_i10", i10), ("d_i11", i11),
                           ("d_y0f", y0f), ("d_wy1", wy1)):
                nc.sync.dma_start(dbg[nm][:], tl[:])
            nc.sync.dma_start(dbg["d_pdots"][:], pdots[:])
            nc.sync.dma_start(dbg["d_grams"][:], grams[:])
        ppo_cm = tc.tile_pool(name="ppo", bufs=1, space=bass.MemorySpace.PSUM)
        ppo = ppo_cm.__enter__()
        ppart = ppo.tile([2, 1], F32, tag="ppart")
        te.matmul(ppart[:], sums2[:], ones1[:])
        psb = cp.tile([2, 1], F32, tag="psb")
        s.copy(psb[:], ppart[:])
        nc.sync.dma_start(out_part[:].rearrange("(a b) -> a b", b=1), psb[:])
        ppo_cm.__exit__(None, None, None)

    return dr


_CACHE = {}


def _build():
    if "nc" not in _CACHE:
        nc = bacc.Bacc(
            "TRN2",
            target_bir_lowering=False,
            debug=False,
            enable_asserts=True,
            num_devices=NCORES,
        )
        with tile.TileContext(nc) as tc:
            _emit(tc)
        nc.compile()
        _CACHE["nc"] = nc
    return _CACHE["nc"]


def _host_inputs(desc1, desc2, homo12, homo21):
    """Per-core input maps (sharding + layout staging only)."""
    f32 = np.float32
    maps = []
    xs = (np.arange(WC, dtype=f32) * GS + 3.5)
    ys = (np.arange(HC, dtype=f32) * GS + 3.5)
    gy, gx = np.meshgrid(ys, xs, indexing="ij")
    cellx = gx.reshape(-1)
    celly = gy.reshape(-1)
    idxg1 = (np.arange(N, dtype=f32) + 1.0).reshape(HC, WC)
    pxv = np.arange(W, dtype=f32).reshape(1, W)
    pyv = np.arange(H, dtype=f32).reshape(1, H)
    ident = np.eye(P, dtype=f32)

    for k in range(NCORES):
        b = k // 4
        lo = (k % 4) * CELLS
        hi = min(lo + CELLS, N)
        nreal = hi - lo
        d1b = np.asarray(desc1[b], dtype=f32).reshape(C, N)
        d1m = np.zeros((C, CELLS), f32)
        d1m[:, :nreal] = d1b[:, lo:hi]
        d1t = np.ascontiguousarray(d1m.T)
        coords = np.zeros((3, CELLS), f32)
        coords[0, :nreal] = cellx[lo:hi]
        coords[1, :nreal] = celly[lo:hi]
        coords[0, nreal:] = 3.5
        coords[1, nreal:] = 3.5
        coords[2, :] = 1.0
        valid = np.zeros(CELLS, f32)
        valid[:nreal] = 1.0
        validm = np.ascontiguousarray(valid.reshape(NB, P).T)
        d2b = np.asarray(desc2[b], dtype=f32).reshape(C, N)
        maps.append({
            "desc2t": np.ascontiguousarray(d2b.T),
            "d1m": d1m,
            "d1t": d1t,
            "coords": coords,
            "homot": np.ascontiguousarray(np.asarray(homo12[b], f32).T),
            "h21": np.ascontiguousarray(np.asarray(homo21[b], f32).reshape(1, 9)),
            "pxv": pxv,
            "pyv": pyv,
            "validm": validm,
            "ident": ident,
            "idxg1": idxg1,
            "onesv": np.ones((1, VCAP), f32),
            "serid": (np.arange(VCAP, dtype=f32).reshape(VCAP // 16, 16).T.copy()),
            "nonce": np.zeros((1, _nonce_len()), f32),
        })
    return maps


def kernel(score1, score2, desc1, desc2, homo12, homo21, _want_trace=False):
    nc = _build()
    maps = _host_inputs(desc1, desc2, homo12, homo21)
    res = run_bass_kernel_spmd(
        nc, maps, list(range(NCORES)), trace=_want_trace
    )
    num = 0.0
    den = 0.0
    for r in res.results:
        p = np.asarray(r["partial"], dtype=np.float64).reshape(-1)
        num += p[0]
        den += p[1]
    out = np.float32(num / den)
    if _want_trace:
        _CACHE["last_exec_time_ns"] = res.exec_time_ns
        _CACHE["last_profile"] = res.profile_json
    return np.array(out, dtype=np.float32)



# revision 18
# speedup vs baseline: 1.5047x; 1.5047x over previous
"""Trainium2 Bass kernel for nn_DenseInterTripletLoss.

Strategy (validated against the reference in numpy first):
  * Shard the 9600 query cells (b=2 x n=4800) across 8 cores: core k handles
    batch k//4, cells [1280*(k%4), ...) padded to 1280 cells (10 blocks of 128).
  * Visibility of image-2 cells is computed exactly via per-cell min/max of the
    5 linear forms {ny+dd, H*dd-ny, nx+dd, W*dd-nx, dd} of the homo21 warp
    (separable over the 8x8 pixel cell), handling denominator sign flips.
  * Only ~136/4800 cells are visible -> the visible column set is compacted on
    device (gpsimd sparse_gather) and the big [rows x 4800] descriptor-distance
    min reduces to a [rows x 160] padded matmul (PE, K=65 with the per-column
    penalty folded in as an extra contraction row).
  * The 4-NN spatial-neighbor exclusion is exact: the top-4 of the separable
    coordinate distance live in a 4x4 window of nearest grid rows/cols
    (j-th row x k-th col nearest requires j*k <= 4). The 4th/5th-smallest
    window distances give a midpoint threshold tau; compacted columns with
    d2 <= tau are penalized out of the min (d2 via a K=4 matmul).
  * Bilinear descriptor sampling: the 4 corner vectors are fetched with one
    gpsimd dma_gather (indices computed on device), pos_sim assembled from
    per-row dot products and the corner Gram matrix.
  * Per-core partial (sum loss*mask, sum mask) pairs are returned; the host
    sums the 8 partials and divides (the unshard step).
"""

import hashlib
import os
import numpy as np

import concourse.bacc as bacc
import concourse.bass as bass
import concourse.mybir as mybir
import concourse.tile as tile
from concourse.bass_utils import run_bass_kernel_spmd
from concourse import library_config

F32 = mybir.dt.float32
I16 = mybir.dt.int16
U32 = mybir.dt.uint32
OP = mybir.AluOpType
AF = mybir.ActivationFunctionType
AX = mybir.AxisListType

H, W, GS = 480, 640, 8
HC, WC = 60, 80
N = HC * WC            # 4800
C = 64
NCORES = 8
CELLS = 1280           # padded cells per core
NB = 10                # blocks of 128 cells
P = 128
VCAP = 160             # compacted visible-column capacity (actual ~136)
KAUG = C + 1
BIGM = 30000.0
EPS = 1e-8
RND = 12582912.0       # 1.5*2^23, round-to-nearest trick (valid for |x| << 2^22)
INV80 = float(np.float32(1.0) / np.float32(80.0))


def _nonce_len():
    """The axon terminal caches compiled executables by an HLO hash that does
    not cover the embedded BIR, so force a distinct HLO whenever the kernel
    source (or build-mode env) changes by varying a dummy input's shape."""
    h = hashlib.sha1()
    with open(os.path.abspath(__file__), "rb") as f:
        h.update(f.read())
    h.update(os.environ.get("KERNEL_STAGE", "").encode())
    h.update(os.environ.get("KERNEL_DEBUG", "").encode())
    h.update(os.environ.get("KERNEL_GDBG", "").encode())
    h.update(os.environ.get("KERNEL_NODOTS", "").encode())
    return 1 + int(h.hexdigest(), 16) % 997


def _emit(tc):
    nc = tc.nc
    dr = {}

    def din(name, shape, dtype=F32):
        dr[name] = nc.dram_tensor(name, list(shape), dtype, kind="ExternalInput").ap()
        return dr[name]

    def dscr(name, shape, dtype=F32):
        dr[name] = nc.dram_tensor(name, list(shape), dtype, kind="Internal").ap()
        return dr[name]

    desc2t = din("desc2t", (N, C))
    d1m = din("d1m", (C, CELLS))
    d1tb = din("d1tb", (P, NB * C))
    coords = din("coords", (3, CELLS))
    homot = din("homot", (3, 3))
    h21 = din("h21", (1, 9))
    pxv = din("pxv", (1, W))
    pyv = din("pyv", (1, H))
    validm = din("validm", (P, NB))
    ident = din("ident", (P, P))
    idxg1 = din("idxg1", (HC, WC))
    onesv = din("onesv", (1, VCAP))
    serid = din("serid", (16, VCAP // 16))
    din("nonce", (1, _nonce_len()))
    out_part = nc.dram_tensor("partial", [2], F32, kind="ExternalOutput").ap()
    DBG = os.environ.get("KERNEL_DEBUG") == "1"
    dbg = {}
    if DBG:
        for nm in ("d_pos", "d_neg", "d_negsq", "d_yp", "d_xp", "d_tau",
                   "d_num", "d_nsq", "d_match", "d_rl",
                   "d_w00", "d_w01", "d_w10", "d_w11", "d_y0f", "d_wy1"):
            dbg[nm] = nc.dram_tensor(nm, [P, NB], F32, kind="ExternalOutput").ap()

    scr_rows = dscr("scr_rows", (4, VCAP))
    scr_widx = dscr("scr_widx", (16, 4 * NB * P // 16), I16)
    scr_vidx = dscr("scr_vidx", (16, VCAP // 16), I16)

    v = nc.vector
    s = nc.scalar
    g = nc.gpsimd
    te = nc.tensor
    STAGE = int(os.environ.get("KERNEL_STAGE", "9"))

    with (
        tc.tile_pool(name="cp", bufs=1) as cp,
        tc.tile_pool(name="wp", bufs=3) as wp,
    ):
        # ---------------- load persistent inputs ----------------
        sb_d1m = cp.tile([C, CELLS], F32, tag="d1m")
        nc.sync.dma_start(sb_d1m[:], d1m[:])
        sb_d1s = cp.tile([C, CELLS], F32, tag="d1s")
        s.mul(sb_d1s[:], sb_d1m[:], -2.0)
        sb_coords = cp.tile([3, CELLS], F32, tag="coords")
        nc.sync.dma_start(sb_coords[:], coords[:])
        sb_homot = cp.tile([3, 3], F32, tag="homot")
        nc.sync.dma_start(sb_homot[:], homot[:])
        sb_h21 = cp.tile([1, 9], F32, tag="h21")
        nc.sync.dma_start(sb_h21[:], h21[:])
        sb_pxv = cp.tile([1, W], F32, tag="pxv")
        nc.sync.dma_start(sb_pxv[:], pxv[:])
        sb_pyv = cp.tile([1, H], F32, tag="pyv")
        nc.sync.dma_start(sb_pyv[:], pyv[:])
        sb_validm = cp.tile([P, NB], F32, tag="validm")
        nc.sync.dma_start(sb_validm[:], validm[:])
        sb_ident = cp.tile([P, P], F32, tag="ident")
        nc.sync.dma_start(sb_ident[:], ident[:])
        sb_idxg1 = cp.tile([HC, WC], F32, tag="idxg1")
        nc.sync.dma_start(sb_idxg1[:], idxg1[:])
        sb_ones = cp.tile([1, VCAP], F32, tag="onesv")
        nc.sync.dma_start(sb_ones[:], onesv[:])
        sb_d1tb = cp.tile([P, NB, C], F32, tag="d1tb")
        nc.sync.dma_start(
            sb_d1tb[:], d1tb[:].rearrange("p (t c) -> p t c", c=C)
        )

        # ---------------- visibility (per image-2 cell) ----------------
        # linear forms L(x,y) = ax*x + (vy(y)); exact per-cell min/max via
        # separable 8-pixel min/max on each axis, combined with a K=2 matmul.
        gg = sb_h21
        e = lambda i, j: gg[0:1, 3 * i + j : 3 * i + j + 1]

        c22 = cp.tile([1, 1], F32, tag="c22")
        v.tensor_single_scalar(c22[:], e(2, 2), EPS, OP.add)
        nyv = cp.tile([1, H], F32, tag="nyv")
        v.tensor_scalar(nyv[:], sb_pyv[:], e(1, 1), e(1, 2), OP.mult, OP.add)
        nxv = cp.tile([1, H], F32, tag="nxv")
        v.tensor_scalar(nxv[:], sb_pyv[:], e(0, 1), e(0, 2), OP.mult, OP.add)
        ddv = cp.tile([1, H], F32, tag="ddv")
        v.tensor_scalar(ddv[:], sb_pyv[:], e(2, 1), c22[:], OP.mult, OP.add)

        # x coefficients
        axA = cp.tile([1, 1], F32, tag="axA")
        v.tensor_tensor(axA[:], e(1, 0), e(2, 0), OP.add)
        axC = cp.tile([1, 1], F32, tag="axC")
        v.tensor_tensor(axC[:], e(0, 0), e(2, 0), OP.add)
        tB = cp.tile([1, 1], F32, tag="tB")
        v.tensor_single_scalar(tB[:], e(2, 0), float(H), OP.mult)
        axB = cp.tile([1, 1], F32, tag="axB")
        v.tensor_tensor(axB[:], tB[:], e(1, 0), OP.subtract)
        tD = cp.tile([1, 1], F32, tag="tD")
        v.tensor_single_scalar(tD[:], e(2, 0), float(W), OP.mult)
        axD = cp.tile([1, 1], F32, tag="axD")
        v.tensor_tensor(axD[:], tD[:], e(0, 0), OP.subtract)

        # y-side vectors
        Av = cp.tile([1, H], F32, tag="Av")
        v.tensor_tensor(Av[:], nyv[:], ddv[:], OP.add)
        Bv = cp.tile([1, H], F32, tag="Bv")
        v.tensor_scalar(Bv[:], ddv[:], float(H), None, OP.mult)
        v.tensor_tensor(Bv[:], Bv[:], nyv[:], OP.subtract)
        Cv = cp.tile([1, H], F32, tag="Cv")
        v.tensor_tensor(Cv[:], nxv[:], ddv[:], OP.add)
        Dv = cp.tile([1, H], F32, tag="Dv")
        v.tensor_scalar(Dv[:], ddv[:], float(W), None, OP.mult)
        v.tensor_tensor(Dv[:], Dv[:], nxv[:], OP.subtract)

        def cell_extreme(ppool, axc, vvec, op, tagp):
            """per-cell extreme (min or max) of axc*px + vvec(py) -> PSUM [60,80]"""
            ux = wp.tile([1, W], F32, tag="ux")
            v.tensor_single_scalar(ux[:], sb_pxv[:], axc[:], OP.mult)
            ue = wp.tile([1, WC], F32, tag="ue")
            v.tensor_reduce(ue[:], ux[:].rearrange("p (a b) -> p a b", b=GS), AX.X, op)
            ve = wp.tile([1, HC], F32, tag="ve")
            v.tensor_reduce(ve[:], vvec[:].rearrange("p (a b) -> p a b", b=GS), AX.X, op)
            ps = ppool.tile([HC, WC], F32, tag=tagp)
            te.matmul(ps[:], ve[:, 0:HC], sb_ones[:, 0:WC], start=True, stop=False)
            te.matmul(ps[:], sb_ones[:, 0:HC], ue[:, 0:WC], start=False, stop=True)
            return ps

        with tc.tile_pool(name="ppv", bufs=3, space=bass.MemorySpace.PSUM) as ppv:
            mins = [
                cell_extreme(ppv, axA, Av, OP.min, "vmin"),
                cell_extreme(ppv, axB, Bv, OP.min, "vmin"),
                cell_extreme(ppv, axC, Cv, OP.min, "vmin"),
                cell_extreme(ppv, axD, Dv, OP.min, "vmin"),
                cell_extreme(ppv, e(2, 0), ddv, OP.min, "vmin"),
            ]
            mpos = cp.tile([HC, WC], F32, tag="mpos")
            s.copy(mpos[:], mins[0][:])
            for t_ in mins[1:]:
                v.tensor_tensor(mpos[:], mpos[:], t_[:], OP.min)
            maxs = [
                cell_extreme(ppv, axA, Av, OP.max, "vmax"),
                cell_extreme(ppv, axB, Bv, OP.max, "vmax"),
                cell_extreme(ppv, axC, Cv, OP.max, "vmax"),
                cell_extreme(ppv, axD, Dv, OP.max, "vmax"),
                cell_extreme(ppv, e(2, 0), ddv, OP.max, "vmax"),
            ]
            mneg = cp.tile([HC, WC], F32, tag="mneg")
            s.copy(mneg[:], maxs[0][:])
            for t_ in maxs[1:]:
                v.tensor_tensor(mneg[:], mneg[:], t_[:], OP.max)
        vpos = cp.tile([HC, WC], F32, tag="vpos")
        v.tensor_single_scalar(vpos[:], mpos[:], 0.0, OP.is_gt)
        vneg = cp.tile([HC, WC], F32, tag="vneg")
        v.tensor_single_scalar(vneg[:], mneg[:], 0.0, OP.is_lt)
        vis01 = cp.tile([HC, WC], F32, tag="vis01")
        v.tensor_tensor(vis01[:], vpos[:], vneg[:], OP.add)
        marker = cp.tile([HC, WC], F32, tag="marker")
        v.tensor_tensor(marker[:], vis01[:], sb_idxg1[:], OP.mult)
        v.tensor_scalar(marker[:], marker[:], 1.0, None, OP.subtract)

        # serialize marker into sparse_gather's [16, 300] wrapped layout,
        # fully on-chip: transpose [60,80]->[80,60] on PE, then 5 strided
        # copies fold partitions (16g+s, r) -> (s, r*5+g).
        sp_in = cp.tile([16, N // 16], F32, tag="sp_in")
        sp_in3 = sp_in[:].rearrange("s (r g) -> s r g", g=5)
        with tc.tile_pool(name="ppsm", bufs=2, space=bass.MemorySpace.PSUM) as ppsm:
            mtp = ppsm.tile([WC, HC], F32, tag="mtp")
            te.transpose(mtp[:], marker[:], sb_ident[0:HC, 0:HC])
            mT = cp.tile([WC, HC], F32, tag="mT")
            s.copy(mT[:], mtp[:])
            # partition fold (16g+s, r) -> (s, r*5+g): PE shift via identity
            for gq in range(5):
                pg = ppsm.tile([16, HC], F32, tag="pg")
                te.matmul(pg[:], sb_ident[0:WC, 16 * gq : 16 * gq + 16], mT[:])
                s.copy(sp_in3[:, :, gq], pg[:])
        sp_out = cp.tile([16, VCAP // 16], F32, tag="sp_out")
        sp_nf = cp.tile([1, 1], U32, tag="sp_nf")
        with tc.tile_critical():
            g.sparse_gather(sp_out[:], sp_in[:], num_found=sp_nf[:])
        sb_serid = cp.tile([16, VCAP // 16], F32, tag="serid")
        nc.sync.dma_start(sb_serid[:], serid[:])
        nf32 = cp.tile([1, 1], F32, tag="nf32")
        v.tensor_copy(nf32[:], sp_nf[:])
        nfp = None
        with tc.tile_pool(name="ppn", bufs=1, space=bass.MemorySpace.PSUM) as ppn:
            nfps = ppn.tile([16, 1], F32, tag="nfps")
            te.matmul(nfps[:], sb_ones[0:1, 0:16], nf32[:])
            nfb = cp.tile([16, 1], F32, tag="nfb")
            s.copy(nfb[:], nfps[:])
        padm = cp.tile([16, VCAP // 16], mybir.dt.uint8, tag="padm")
        v.tensor_single_scalar(padm[:], sb_serid[:], nfb[:], OP.is_lt)
        spc = cp.tile([16, VCAP // 16], F32, tag="spc")
        v.memset(spc[:], -1.0)
        v.copy_predicated(spc[:], padm[:], sp_out[:])

        # ---------------- compacted visible columns ----------------
        sp0 = cp.tile([16, VCAP // 16], F32, tag="sp0")
        v.tensor_single_scalar(sp0[:], spc[:], 0.0, OP.max)
        idxs16a = cp.tile([16, VCAP // 16], I16, tag="idxs16a")
        v.tensor_copy(idxs16a[:], sp0[:])
        nc.sync.dma_start(scr_vidx[:], idxs16a[:])
        idxs16r = cp.tile([P, VCAP // 16], I16, tag="idxs16r")
        for rr_ in range(8):
            nc.sync.dma_start(idxs16r[16 * rr_ : 16 * rr_ + 16, :], scr_vidx[:])
        idxs16 = cp.tile([P, VCAP // 16], I16, tag="idxs16")
        v.tensor_copy(idxs16[:], idxs16r[:])

        visraw = cp.tile([P, 2, C], F32, tag="visraw")
        gsem = nc.alloc_semaphore("gsem")
        with tc.tile_critical():
            g.dma_gather(
                visraw[:], desc2t[:], idxs16[:],
                num_idxs=VCAP, num_idxs_reg=VCAP, elem_size=C,
            ).then_inc(gsem, 16)
            g.wait_ge(gsem, 16)

        sb_rhsS = cp.tile([KAUG, VCAP], F32, tag="rhsS")
        with tc.tile_pool(name="ppt", bufs=2, space=bass.MemorySpace.PSUM) as ppt:
            for q in range(2):
                pt = ppt.tile([C, P], F32, tag="ptr")
                te.transpose(pt[:], visraw[:, q, :], sb_ident[:])
                w_ = P if q == 0 else VCAP - P
                s.copy(sb_rhsS[0:C, q * P : q * P + w_], pt[:, 0:w_])

        # penalty row: 2 for valid slots, 2+BIGM for pads (sparse pads are -1)
        r64 = cp.tile([16, VCAP // 16], F32, tag="r64")
        v.tensor_scalar(r64[:], spc[:], 0.0, BIGM, OP.is_lt, OP.mult)
        v.tensor_scalar(r64[:], r64[:], 2.0, None, OP.add)
        nc.sync.dma_start(
            scr_rows[0:1, :].rearrange("o (f p) -> o p f", p=16), r64[:]
        )
        nc.sync.dma_start(sb_rhsS[C : C + 1, :], scr_rows[0:1, :])

        # d2c rhs rows {-2yj, -2xj, 1, b2j} derived from compacted indices
        uu = cp.tile([16, VCAP // 16], F32, tag="uu")
        v.tensor_scalar(uu[:], spc[:], 0.5, INV80, OP.add, OP.mult)
        rr0 = cp.tile([16, VCAP // 16], F32, tag="rr0")
        v.tensor_scalar(rr0[:], uu[:], RND, RND, OP.add, OP.subtract)
        rgt = cp.tile([16, VCAP // 16], F32, tag="rgt")
        v.tensor_tensor(rgt[:], rr0[:], uu[:], OP.is_gt)
        rj = cp.tile([16, VCAP // 16], F32, tag="rj")
        v.tensor_tensor(rj[:], rr0[:], rgt[:], OP.subtract)
        cj = cp.tile([16, VCAP // 16], F32, tag="cj")
        v.tensor_scalar(cj[:], rj[:], 80.0, None, OP.mult)
        v.tensor_tensor(cj[:], spc[:], cj[:], OP.subtract)
        rw0 = cp.tile([16, VCAP // 16], F32, tag="rw0")
        v.tensor_scalar(rw0[:], rj[:], -16.0, -7.0, OP.mult, OP.add)
        rw1 = cp.tile([16, VCAP // 16], F32, tag="rw1")
        v.tensor_scalar(rw1[:], cj[:], -16.0, -7.0, OP.mult, OP.add)
        yj = cp.tile([16, VCAP // 16], F32, tag="yj")
        v.tensor_scalar(yj[:], rj[:], 8.0, 3.5, OP.mult, OP.add)
        xj = cp.tile([16, VCAP // 16], F32, tag="xj")
        v.tensor_scalar(xj[:], cj[:], 8.0, 3.5, OP.mult, OP.add)
        b2j = cp.tile([16, VCAP // 16], F32, tag="b2j")
        v.tensor_tensor(b2j[:], yj[:], yj[:], OP.mult)
        xj2 = cp.tile([16, VCAP // 16], F32, tag="xj2")
        v.tensor_tensor(xj2[:], xj[:], xj[:], OP.mult)
        v.tensor_tensor(b2j[:], b2j[:], xj2[:], OP.add)

        sb_d2crhs = cp.tile([4, VCAP], F32, tag="d2crhs")
        for i, tl in ((1, rw0), (2, rw1), (3, b2j)):
            nc.sync.dma_start(
                scr_rows[i : i + 1, :].rearrange("o (f p) -> o p f", p=16), tl[:]
            )
        nc.sync.dma_start(sb_d2crhs[0:1, :], scr_rows[1:2, :])
        nc.sync.dma_start(sb_d2crhs[1:2, :], scr_rows[2:3, :])
        nc.sync.dma_start(sb_d2crhs[2:3, :], onesv[:])
        nc.sync.dma_start(sb_d2crhs[3:4, :], scr_rows[3:4, :])

        if STAGE <= 1:
            ones1 = cp.tile([P, 1], F32, tag="ones1")
            v.memset(ones1[:], 1.0)
            sums2 = cp.tile([P, 2], F32, tag="sums2")
            v.tensor_copy(sums2[0:KAUG, 0:1], sb_rhsS[:, 0:1])
            v.tensor_copy(sums2[0:4, 1:2], sb_d2crhs[:, 0:1])
            psb = cp.tile([2, 1], F32, tag="psb")
            v.tensor_copy(psb[:], sums2[0:2, 0:1])
            nc.sync.dma_start(out_part[:].rearrange("(a b) -> a b", b=1), psb[:])
            return dr

        # ---------------- warp (both layouts) ----------------
        yp = cp.tile([P, NB], F32, tag="yp")
        xp = cp.tile([P, NB], F32, tag="xp")
        lhsTd2c = []
        ppw_cm = tc.tile_pool(name="ppw", bufs=2, space=bass.MemorySpace.PSUM)
        ppw = ppw_cm.__enter__()
        for t in range(NB):
            qp = ppw.tile([P, 3], F32, tag="qp")
            te.matmul(qp[:], sb_coords[:, bass.ts(t, P)], sb_homot[:])
            zc = wp.tile([P, 1], F32, tag="zc")
            v.tensor_scalar(zc[:], qp[:, 2:3], EPS, None, OP.add)
            rz = wp.tile([P, 1], F32, tag="rz")
            v.reciprocal(rz[:], zc[:])
            v.tensor_tensor(yp[:, t : t + 1], qp[:, 1:2], rz[:], OP.mult)
            v.tensor_tensor(xp[:, t : t + 1], qp[:, 0:1], rz[:], OP.mult)

            quad = wp.tile([P, 4], F32, tag="quad")
            v.tensor_copy(quad[:, 0:1], yp[:, t : t + 1])
            v.tensor_copy(quad[:, 1:2], xp[:, t : t + 1])
            v.tensor_tensor(quad[:, 2:3], yp[:, t : t + 1], yp[:, t : t + 1], OP.mult)
            x2c = wp.tile([P, 1], F32, tag="x2c")
            v.tensor_tensor(x2c[:], xp[:, t : t + 1], xp[:, t : t + 1], OP.mult)
            v.tensor_tensor(quad[:, 2:3], quad[:, 2:3], x2c[:], OP.add)
            v.memset(quad[:, 3:4], 1.0)
            ldp = ppw.tile([4, P], F32, tag="qk")
            te.transpose(ldp[:], quad[:], sb_ident[:])
            ld = cp.tile([4, P], F32, tag=f"lhsTd2c{t}")
            s.copy(ld[:], ldp[:])
            lhsTd2c.append(ld)
        ppw_cm.__exit__(None, None, None)

        if STAGE <= 2:
            sums2 = cp.tile([P, 2], F32, tag="sums2")
            v.tensor_copy(sums2[:, 0:1], yp[:, 0:1])
            v.tensor_copy(sums2[:, 1:2], xp[:, 0:1])
            psb = cp.tile([2, 1], F32, tag="psb")
            v.tensor_copy(psb[:], sums2[0:2, 0:1])
            nc.sync.dma_start(out_part[:].rearrange("(a b) -> a b", b=1), psb[:])
            return dr

        # ---------------- per-cell floors / windows / weights ----------------
        def floor_of(src, tag):
            r0 = cp.tile([P, NB], F32, tag=tag + "_r0")
            v.tensor_scalar(r0[:], src[:], RND, RND, OP.add, OP.subtract)
            gt_ = cp.tile([P, NB], F32, tag=tag + "_gt")
            v.tensor_tensor(gt_[:], r0[:], src[:], OP.is_gt)
            fl = cp.tile([P, NB], F32, tag=tag)
            v.tensor_tensor(fl[:], r0[:], gt_[:], OP.subtract)
            return fl

        dgy = cp.tile([P, NB], F32, tag="dgy")
        v.tensor_scalar(dgy[:], yp[:], -4.0, 0.5, OP.add, OP.add)
        v.tensor_scalar(dgy[:], dgy[:], 0.125, None, OP.mult)
        dgx = cp.tile([P, NB], F32, tag="dgx")
        v.tensor_scalar(dgx[:], xp[:], -4.0, 0.5, OP.add, OP.add)
        v.tensor_scalar(dgx[:], dgx[:], 0.125, None, OP.mult)

        y0f = floor_of(dgy, "y0f")
        x0f = floor_of(dgx, "x0f")
        wy1 = cp.tile([P, NB], F32, tag="wy1")
        v.tensor_tensor(wy1[:], dgy[:], y0f[:], OP.subtract)
        wx1 = cp.tile([P, NB], F32, tag="wx1")
        v.tensor_tensor(wx1[:], dgx[:], x0f[:], OP.subtract)

        def clampf(src, lo, hi, tag):
            t_ = cp.tile([P, NB], F32, tag=tag)
            v.tensor_scalar(t_[:], src[:], lo, hi, OP.max, OP.min)
            return t_

        y0c = clampf(y0f, 0.0, float(HC - 1), "y0c")
        x0c = clampf(x0f, 0.0, float(WC - 1), "x0c")
        y1f = cp.tile([P, NB], F32, tag="y1f")
        v.tensor_scalar(y1f[:], y0f[:], 1.0, None, OP.add)
        x1f = cp.tile([P, NB], F32, tag="x1f")
        v.tensor_scalar(x1f[:], x0f[:], 1.0, None, OP.add)
        y1c = clampf(y1f, 0.0, float(HC - 1), "y1c")
        x1c = clampf(x1f, 0.0, float(WC - 1), "x1c")

        def iseq(a, b, tag):
            t_ = cp.tile([P, NB], F32, tag=tag)
            v.tensor_tensor(t_[:], a[:], b[:], OP.is_equal)
            return t_

        vy0 = iseq(y0f, y0c, "vy0")
        vy1 = iseq(y1f, y1c, "vy1")
        vx0 = iseq(x0f, x0c, "vx0")
        vx1 = iseq(x1f, x1c, "vx1")

        w4r = cp.tile([P, NB], F32, tag="w4r")
        v.tensor_scalar(w4r[:], y0f[:], -1.0, 0.0, OP.add, OP.max)
        v.tensor_scalar(w4r[:], w4r[:], float(HC - 4), None, OP.min)
        w4c = cp.tile([P, NB], F32, tag="w4c")
        v.tensor_scalar(w4c[:], x0f[:], -1.0, 0.0, OP.add, OP.max)
        v.tensor_scalar(w4c[:], w4c[:], float(WC - 4), None, OP.min)

        # negated squared axis distances for the 4x4 window
        ndy2 = cp.tile([P, NB, 4], F32, tag="ndy2")
        ndx2 = cp.tile([P, NB, 4], F32, tag="ndx2")
        for a in range(4):
            ty = wp.tile([P, NB], F32, tag="wty")
            v.tensor_scalar(ty[:], w4r[:], 8.0, 3.5 + 8.0 * a, OP.mult, OP.add)
            v.tensor_tensor(ty[:], yp[:], ty[:], OP.subtract)
            v.tensor_tensor(ty[:], ty[:], ty[:], OP.mult)
            v.tensor_scalar(ndy2[:, :, a], ty[:], -1.0, None, OP.mult)
            tx = wp.tile([P, NB], F32, tag="wtx")
            v.tensor_scalar(tx[:], w4c[:], 8.0, 3.5 + 8.0 * a, OP.mult, OP.add)
            v.tensor_tensor(tx[:], xp[:], tx[:], OP.subtract)
            v.tensor_tensor(tx[:], tx[:], tx[:], OP.mult)
            v.tensor_scalar(ndx2[:, :, a], tx[:], -1.0, None, OP.mult)
        nd2w = cp.tile([P, NB, 16], F32, tag="nd2w")
        for a in range(4):
            for b in range(4):
                v.tensor_tensor(
                    nd2w[:, :, 4 * a + b], ndy2[:, :, a], ndx2[:, :, b], OP.add
                )

        tau = cp.tile([P, NB], F32, tag="tau")
        for t in range(NB):
            m8 = wp.tile([P, 8], F32, tag="m8")
            v.max(m8[:], nd2w[:, t, :])
            s34 = wp.tile([P, 1], F32, tag="s34")
            v.tensor_tensor(s34[:], m8[:, 3:4], m8[:, 4:5], OP.add)
            v.tensor_scalar(tau[:, t : t + 1], s34[:], -0.5, None, OP.mult)

        if STAGE <= 3:
            sums2 = cp.tile([P, 2], F32, tag="sums2")
            v.tensor_copy(sums2[:, 0:1], tau[:, 0:1])
            v.tensor_copy(sums2[:, 1:2], y0f[:, 0:1])
            psb = cp.tile([2, 1], F32, tag="psb")
            v.tensor_copy(psb[:], sums2[0:2, 0:1])
            nc.sync.dma_start(out_part[:].rearrange("(a b) -> a b", b=1), psb[:])
            return dr

        # ---------------- corner indices -> gather ----------------
        # gather element j = (4t+k)*128 + p must sit at wrapped idx slot
        # (row j%16 = p%16, col j//16 = (4t+k)*8 + p//16). Build the wrap
        # on-chip with PE transposes instead of 46k 2-byte DMA descriptors.
        ty0 = cp.tile([P, NB], F32, tag="ty0")
        v.tensor_scalar(ty0[:], y0c[:], 80.0, None, OP.mult)
        ty1 = cp.tile([P, NB], F32, tag="ty1")
        v.tensor_scalar(ty1[:], y1c[:], 80.0, None, OP.mult)
        cidx4 = cp.tile([P, NB, 4], F32, tag="cidx4")
        v.tensor_tensor(cidx4[:, :, 0], ty0[:], x0c[:], OP.add)
        v.tensor_tensor(cidx4[:, :, 1], ty0[:], x1c[:], OP.add)
        v.tensor_tensor(cidx4[:, :, 2], ty1[:], x0c[:], OP.add)
        v.tensor_tensor(cidx4[:, :, 3], ty1[:], x1c[:], OP.add)

        NQ = 4 * NB  # 40 (t,k) slots
        wfull = cp.tile([16, NQ * 8], F32, tag="wfull")
        wf3 = wfull[:].rearrange("r (c q) -> r c q", q=8)
        with tc.tile_pool(name="ppx", bufs=2, space=bass.MemorySpace.PSUM) as ppx:
            t1p = ppx.tile([NQ, P], F32, tag="t1p")
            te.transpose(t1p[:], cidx4[:].rearrange("p t k -> p (t k)"), sb_ident[:])
            ct = cp.tile([NQ, P], F32, tag="ct")
            s.copy(ct[:], t1p[:])
            for q in range(8):
                t2p = ppx.tile([16, NQ], F32, tag="t2p")
                te.transpose(t2p[:], ct[:, 16 * q : 16 * q + 16],
                             sb_ident[0:NQ, 0:NQ])
                s.copy(wf3[:, :, q], t2p[:])
        wi16 = cp.tile([16, NQ * 8], I16, tag="wi16")
        v.tensor_copy(wi16[:], wfull[:])
        nc.sync.dma_start(scr_widx[:], wi16[:])
        cgidx = cp.tile([P, 4 * P * NB // 16], I16, tag="cgidx")
        for rr_ in range(8):
            nc.sync.dma_start(cgidx[16 * rr_ : 16 * rr_ + 16, :], scr_widx[:])
        if STAGE <= 4:
            d_cg = nc.dram_tensor("d_cgidx", [P, 4 * P * NB // 16], I16,
                                  kind="ExternalOutput").ap()
            nc.sync.dma_start(d_cg[:], cgidx[:])
            cgf = cp.tile([16, 4 * P * NB // 16], F32, tag="cgf")
            v.tensor_copy(cgf[:], cgidx[0:16, :])
            sums2 = cp.tile([P, 2], F32, tag="sums2")
            v.tensor_copy(sums2[0:16, 0:1], cgf[:, 0:1])
            v.tensor_copy(sums2[0:16, 1:2], cgf[:, 1:2])
            psb = cp.tile([2, 1], F32, tag="psb")
            v.tensor_copy(psb[:], sums2[0:2, 0:1])
            nc.sync.dma_start(out_part[:].rearrange("(a b) -> a b", b=1), psb[:])
            return dr

        if os.environ.get("KERNEL_GDBG") == "zero":
            v.memset(cgidx[:], 0)
        visCorn = cp.tile([P, 4 * NB, C], F32, tag="visCorn")
        gsem2 = nc.alloc_semaphore("gsem2")
        NSPLIT = 8
        QS = 4 * NB // NSPLIT
        with tc.tile_critical():
            for j in range(NSPLIT):
                g.dma_gather(
                    visCorn[:, QS * j : QS * (j + 1), :], desc2t[:],
                    cgidx[:, QS * 8 * j : QS * 8 * (j + 1)],
                    num_idxs=P * QS, num_idxs_reg=P * QS, elem_size=C,
                ).then_inc(gsem2, 16)
            g.wait_ge(gsem2, 16 * NSPLIT)

        if STAGE <= 5:
            sums2 = cp.tile([P, 2], F32, tag="sums2")
            v.tensor_copy(sums2[:, 0:1], visCorn[:, 0, 0:1])
            v.tensor_copy(sums2[:, 1:2], tau[:, 0:1])
            psb = cp.tile([2, 1], F32, tag="psb")
            v.tensor_copy(psb[:], sums2[0:2, 0:1])
            nc.sync.dma_start(out_part[:].rearrange("(a b) -> a b", b=1), psb[:])
            return dr

        # ---------------- bilinear corner weights (needed in main loop) ----
        u1 = cp.tile([P, NB], F32, tag="u1")  # 1-wy1
        v.tensor_scalar(u1[:], wy1[:], -1.0, 1.0, OP.mult, OP.add)
        u2 = cp.tile([P, NB], F32, tag="u2")  # 1-wx1
        v.tensor_scalar(u2[:], wx1[:], -1.0, 1.0, OP.mult, OP.add)

        # corner weights w_k = wgt_y * wgt_x * valid_y * valid_x
        def mkw(wy_, vy_, wx_, vx_, tag):
            t_ = cp.tile([P, NB], F32, tag=tag)
            v.tensor_tensor(t_[:], wy_[:], wx_[:], OP.mult)
            vv = wp.tile([P, NB], F32, tag="vv")
            v.tensor_tensor(vv[:], vy_[:], vx_[:], OP.mult)
            v.tensor_tensor(t_[:], t_[:], vv[:], OP.mult)
            return t_

        w00 = mkw(u1, vy0, u2, vx0, "w00")
        w01 = mkw(u1, vy0, wx1, vx1, "w01")
        w10 = mkw(wy1, vy1, u2, vx0, "w10")
        w11 = mkw(wy1, vy1, wx1, vx1, "w11")

        # ---------------- main per-block loop ----------------
        negsq = cp.tile([P, NB], F32, tag="negsq")
        num = cp.tile([P, NB], F32, tag="num")
        nsq = cp.tile([P, NB], F32, tag="nsq")

        ppm_cm = tc.tile_pool(name="ppm", bufs=3, space=bass.MemorySpace.PSUM)
        ppm = ppm_cm.__enter__()
        for t in range(NB):
            lhsTS = wp.tile([KAUG, P], F32, tag="lhsTS")
            s.copy(lhsTS[0:C, :], sb_d1s[:, bass.ts(t, P)])
            v.memset(lhsTS[C : C + 1, :], 1.0)
            tps = ppm.tile([P, VCAP], F32, tag="tps")
            te.matmul(tps[:], lhsTS[:], sb_rhsS[:])
            dps = ppm.tile([P, VCAP], F32, tag="dps")
            te.matmul(dps[:], lhsTd2c[t][:], sb_d2crhs[:])
            pen = wp.tile([P, VCAP], F32, tag="pen")
            v.tensor_scalar(pen[:], dps[:], tau[:, t : t + 1], BIGM, OP.is_le, OP.mult)
            tfin = wp.tile([P, VCAP], F32, tag="tfin")
            v.tensor_tensor(tfin[:], tps[:], pen[:], OP.add)
            v.tensor_reduce(negsq[:, t : t + 1], tfin[:], AX.X, OP.min)

            # w_desc1 = sum_k w_k * corner_k, then num = d1.wd, nsq = |wd|^2
            wd = wp.tile([P, C], F32, tag="wd")
            tk = wp.tile([P, C], F32, tag="tk")
            v.tensor_scalar(wd[:], visCorn[:, 4 * t + 0, :], w00[:, t : t + 1],
                            None, OP.mult)
            v.tensor_scalar(tk[:], visCorn[:, 4 * t + 1, :], w01[:, t : t + 1],
                            None, OP.mult)
            v.tensor_tensor(wd[:], wd[:], tk[:], OP.add)
            v.tensor_scalar(tk[:], visCorn[:, 4 * t + 2, :], w10[:, t : t + 1],
                            None, OP.mult)
            v.tensor_tensor(wd[:], wd[:], tk[:], OP.add)
            v.tensor_scalar(tk[:], visCorn[:, 4 * t + 3, :], w11[:, t : t + 1],
                            None, OP.mult)
            v.tensor_tensor(wd[:], wd[:], tk[:], OP.add)
            v.tensor_tensor(tk[:], wd[:], wd[:], OP.mult)
            v.tensor_reduce(nsq[:, t : t + 1], tk[:], AX.X, OP.add)
            v.tensor_tensor(tk[:], wd[:], sb_d1tb[:, t, :], OP.mult)
            v.tensor_reduce(num[:, t : t + 1], tk[:], AX.X, OP.add)
        ppm_cm.__exit__(None, None, None)

        if STAGE <= 6:
            sums2 = cp.tile([P, 2], F32, tag="sums2")
            v.tensor_copy(sums2[:, 0:1], negsq[:, 0:1])
            v.tensor_copy(sums2[:, 1:2], num[:, 0:1])
            psb = cp.tile([2, 1], F32, tag="psb")
            v.tensor_copy(psb[:], sums2[0:2, 0:1])
            nc.sync.dma_start(out_part[:].rearrange("(a b) -> a b", b=1), psb[:])
            return dr

        # ---------------- pos / neg / loss ----------------
        v.tensor_scalar(nsq[:], nsq[:], 0.0, None, OP.max)
        nrm = cp.tile([P, NB], F32, tag="nrm")
        s.sqrt(nrm[:], nsq[:])
        v.tensor_scalar(nrm[:], nrm[:], EPS, None, OP.add)
        rden = cp.tile([P, NB], F32, tag="rden")
        v.reciprocal(rden[:], nrm[:])
        posd = cp.tile([P, NB], F32, tag="posd")
        v.tensor_tensor(posd[:], num[:], rden[:], OP.mult)
        pa = cp.tile([P, NB], F32, tag="pa")
        v.tensor_scalar(pa[:], posd[:], -2.0, 2.0, OP.mult, OP.add)
        v.tensor_scalar(pa[:], pa[:], EPS, None, OP.max)
        pos = cp.tile([P, NB], F32, tag="pos")
        s.sqrt(pos[:], pa[:])

        ngc = cp.tile([P, NB], F32, tag="ngc")
        v.tensor_scalar(ngc[:], negsq[:], EPS, None, OP.max)
        neg = cp.tile([P, NB], F32, tag="neg")
        s.sqrt(neg[:], ngc[:])

        # match mask
        m1 = cp.tile([P, NB], F32, tag="m1")
        v.tensor_single_scalar(m1[:], yp[:], 0.0, OP.is_ge)
        m2 = cp.tile([P, NB], F32, tag="m2")
        v.tensor_single_scalar(m2[:], yp[:], float(H - 1), OP.is_le)
        v.tensor_tensor(m1[:], m1[:], m2[:], OP.mult)
        v.tensor_single_scalar(m2[:], xp[:], 0.0, OP.is_ge)
        v.tensor_tensor(m1[:], m1[:], m2[:], OP.mult)
        v.tensor_single_scalar(m2[:], xp[:], float(W - 1), OP.is_le)
        v.tensor_tensor(m1[:], m1[:], m2[:], OP.mult)
        v.tensor_tensor(m1[:], m1[:], sb_validm[:], OP.mult)

        lt = cp.tile([P, NB], F32, tag="lt")
        v.tensor_tensor(lt[:], pos[:], neg[:], OP.subtract)
        rl = cp.tile([P, NB], F32, tag="rl")
        s.activation(rl[:], lt[:], AF.Relu, bias=1.0, scale=1.0)
        rm = cp.tile([P, NB], F32, tag="rm")
        v.tensor_tensor(rm[:], rl[:], m1[:], OP.mult)

        sums2 = cp.tile([P, 2], F32, tag="sums2")
        lsc = cp.tile([P, NB], F32, tag="lsc")
        v.tensor_tensor(lsc[:], rm[:], rl[:], OP.mult)
        v.tensor_reduce(sums2[:, 0:1], lsc[:], AX.X, OP.add)
        v.tensor_reduce(sums2[:, 1:2], m1[:], AX.X, OP.add)
        ones1 = cp.tile([P, 1], F32, tag="ones1")
        v.memset(ones1[:], 1.0)
        if DBG:
            for nm, tl in (("d_pos", pos), ("d_neg", neg), ("d_negsq", negsq),
                           ("d_yp", yp), ("d_xp", xp), ("d_tau", tau),
                           ("d_num", num), ("d_nsq", nsq), ("d_match", m1),
                           ("d_rl", rl), ("d_w00", w00), ("d_w01", w01),
                           ("d_w10", w10), ("d_w11", w11),
                           ("d_y0f", y0f), ("d_wy1", wy1)):
                nc.sync.dma_start(dbg[nm][:], tl[:])
        ppo_cm = tc.tile_pool(name="ppo", bufs=1, space=bass.MemorySpace.PSUM)
        ppo = ppo_cm.__enter__()
        ppart = ppo.tile([2, 1], F32, tag="ppart")
        te.matmul(ppart[:], sums2[:], ones1[:])
        psb = cp.tile([2, 1], F32, tag="psb")
        s.copy(psb[:], ppart[:])
        nc.sync.dma_start(out_part[:].rearrange("(a b) -> a b", b=1), psb[:])
        ppo_cm.__exit__(None, None, None)

    return dr


_CACHE = {}


def _build():
    if "nc" not in _CACHE:
        nc = bacc.Bacc(
            "TRN2",
            target_bir_lowering=False,
            debug=False,
            enable_asserts=True,
            num_devices=NCORES,
        )
        with tile.TileContext(nc) as tc:
            _emit(tc)
        nc.compile()
        _CACHE["nc"] = nc
    return _CACHE["nc"]


def _host_inputs(desc1, desc2, homo12, homo21):
    """Per-core input maps (sharding + layout staging only)."""
    f32 = np.float32
    maps = []
    xs = (np.arange(WC, dtype=f32) * GS + 3.5)
    ys = (np.arange(HC, dtype=f32) * GS + 3.5)
    gy, gx = np.meshgrid(ys, xs, indexing="ij")
    cellx = gx.reshape(-1)
    celly = gy.reshape(-1)
    idxg1 = (np.arange(N, dtype=f32) + 1.0).reshape(HC, WC)
    pxv = np.arange(W, dtype=f32).reshape(1, W)
    pyv = np.arange(H, dtype=f32).reshape(1, H)
    ident = np.eye(P, dtype=f32)

    for k in range(NCORES):
        b = k // 4
        lo = (k % 4) * CELLS
        hi = min(lo + CELLS, N)
        nreal = hi - lo
        d1b = np.asarray(desc1[b], dtype=f32).reshape(C, N)
        d1m = np.zeros((C, CELLS), f32)
        d1m[:, :nreal] = d1b[:, lo:hi]
        # [P, NB*C] with row p = cells {t*128+p}: avoids strided-DMA staging
        d1tb = np.ascontiguousarray(
            d1m.T.reshape(NB, P, C).transpose(1, 0, 2).reshape(P, NB * C)
        )
        coords = np.zeros((3, CELLS), f32)
        coords[0, :nreal] = cellx[lo:hi]
        coords[1, :nreal] = celly[lo:hi]
        coords[0, nreal:] = 3.5
        coords[1, nreal:] = 3.5
        coords[2, :] = 1.0
        valid = np.zeros(CELLS, f32)
        valid[:nreal] = 1.0
        validm = np.ascontiguousarray(valid.reshape(NB, P).T)
        d2b = np.asarray(desc2[b], dtype=f32).reshape(C, N)
        maps.append({
            "desc2t": np.ascontiguousarray(d2b.T),
            "d1m": d1m,
            "d1tb": d1tb,
            "coords": coords,
            "homot": np.ascontiguousarray(np.asarray(homo12[b], f32).T),
            "h21": np.ascontiguousarray(np.asarray(homo21[b], f32).reshape(1, 9)),
            "pxv": pxv,
            "pyv": pyv,
            "validm": validm,
            "ident": ident,
            "idxg1": idxg1,
            "onesv": np.ones((1, VCAP), f32),
            "serid": (np.arange(VCAP, dtype=f32).reshape(VCAP // 16, 16).T.copy()),
            "nonce": np.zeros((1, _nonce_len()), f32),
        })
    return maps


def kernel(score1, score2, desc1, desc2, homo12, homo21, _want_trace=False):
    nc = _build()
    maps = _host_inputs(desc1, desc2, homo12, homo21)
    res = run_bass_kernel_spmd(
        nc, maps, list(range(NCORES)), trace=_want_trace
    )
    num = 0.0
    den = 0.0
    for r in res.results:
        p = np.asarray(r["partial"], dtype=np.float64).reshape(-1)
        num += p[0]
        den += p[1]
    out = np.float32(num / den)
    if _want_trace:
        _CACHE["last_exec_time_ns"] = res.exec_time_ns
        _CACHE["last_profile"] = res.profile_json
    return np.array(out, dtype=np.float32)

